# revision 34
# baseline (speedup 1.0000x reference)
"""Trainium2 Bass kernel for nn_Actor_77171972374917 (GIN message passing actor).

Strategy:
- Data parallel: 512 graphs -> 64 graphs on each of 8 NeuronCores.
- Device computes the heavy part: 3 GIN conv layers on the dense [256,256]
  adjacency (reads the int32 states, all matmuls fp32) and emits the final
  node embeddings x^T [128,256] per graph plus row/col sums.
- conv1 exploits all-ones input features: MLP input is (1+rowsum) ⊗ onesW1,
  a rank-1 matmul -- no adjacency matmul needed for layer 1.
- Host (numpy fp32) runs the tiny heads: masked softmaxes, Gumbel-argmax
  sampling (jax.random key 42 Gumbel noise precomputed on CPU).
"""

import numpy as np

G, N, F_IN, E = 512, 256, 32, 128
NCORES = 8
GPC = G // NCORES  # 64 graphs per core

_cache = {}


def _build_bass(gpc=GPC):
    import concourse.bass as bass
    from concourse import bacc
    import concourse.mybir as mybir
    from concourse.tile import TileContext
    from concourse.masks import make_identity

    fp32 = mybir.dt.float32
    bf16 = mybir.dt.bfloat16
    i32 = mybir.dt.int32

    nc = bacc.Bacc()
    states_p = nc.declare_dram_parameter("states", (gpc, N, N), i32, isOutput=False)
    wts_p = nc.declare_dram_parameter("wts", (128, 5 * 128 + 6), fp32, isOutput=False)
    s1_p = nc.declare_dram_parameter("s1row", (1, 128), fp32, isOutput=False)
    ones_p = nc.declare_dram_parameter("ones", (128, 128), fp32, isOutput=False)
    xt_p = nc.declare_dram_parameter("xt", (gpc, 128, N), fp32, isOutput=True)
    sums_p = nc.declare_dram_parameter("sums", (gpc, 2, N), fp32, isOutput=True)

    B = 4  # graphs interleaved per stage (hides PE<->ACT ping-pong)
    with TileContext(nc) as tc:
        with tc.tile_pool(name="const", bufs=1) as const, \
             tc.tile_pool(name="adji", bufs=gpc) as adjip, \
             tc.tile_pool(name="adj", bufs=2 * B) as adjp, \
             tc.tile_pool(name="work", bufs=2 * B) as work, \
             tc.tile_pool(name="ps", bufs=4, space="PSUM") as ps, \
             tc.tile_pool(name="pst", bufs=2, space="PSUM") as pst:

            wts = const.tile([128, 5 * 128 + 6], fp32)
            nc.sync.dma_start(out=wts, in_=wts_p[:, :])
            s1row = const.tile([1, 128], fp32)
            nc.sync.dma_start(out=s1row, in_=s1_p[:, :])
            ones_col = const.tile([128, 128], fp32)
            nc.sync.dma_start(out=ones_col, in_=ones_p[:, :])
            identf = const.tile([128, 128], fp32)
            make_identity(nc, identf)
            ones_bf = const.tile([128, 128], bf16)
            nc.vector.tensor_copy(ones_bf, ones_col)

            W = lambda i: wts[:, 128 * i:128 * (i + 1)]
            Wb = lambda i: wts[:, 5 * 128 + i:5 * 128 + i + 1]
            # weight cols: 0=W2c1 1=W1c2 2=W2c2 3=W1c3 4=W2c3
            # bias cols: 0=b1c1 1=b2c1 2=b1c2 3=b2c2 4=b1c3 5=b2c3

            for blk in range(gpc // B):
                gs = range(blk * B, (blk + 1) * B)
                adjs, degs, css, rss, xts = {}, {}, {}, {}, {}
                for g in gs:
                    adj_i = adjip.tile([128, 2, N], i32, tag="adj_i")
                    nc.gpsimd.dma_start(
                        out=adj_i,
                        in_=states_p[g].rearrange("(c p) n -> p c n", p=128))
                    adj = adjp.tile([128, 2, N], bf16, tag="adj_f")
                    nc.vector.tensor_copy(adj, adj_i)
                    adjs[g] = adj
                for g in gs:
                    deg = work.tile([128, 2], fp32, tag="deg")
                    nc.vector.reduce_sum(deg, adjs[g], axis=mybir.AxisListType.X)
                    degs[g] = deg
                for g in gs:
                    cs_ps = ps.tile([128, N], fp32, tag="mm")
                    nc.tensor.matmul(cs_ps, lhsT=ones_bf, rhs=adjs[g][:, 0], start=True, stop=False)
                    nc.tensor.matmul(cs_ps, lhsT=ones_bf, rhs=adjs[g][:, 1], start=False, stop=True)
                    css[g] = cs_ps
                for g in gs:
                    csrow = work.tile([1, N], fp32, tag="csrow")
                    nc.vector.tensor_copy(csrow, css[g][0:1])
                    nc.scalar.dma_start(out=sums_p[g, 1:2], in_=csrow)
                for g in gs:
                    degT0 = pst.tile([1, 128], fp32, tag="degT")
                    degT1 = pst.tile([1, 128], fp32, tag="degT")
                    nc.tensor.transpose(degT0, degs[g][:, 0:1], identf)
                    nc.tensor.transpose(degT1, degs[g][:, 1:2], identf)
                    rs = work.tile([1, 2, 128], fp32, tag="rs")
                    nc.scalar.add(rs[:, 0], degT0, 1.0)
                    nc.scalar.add(rs[:, 1], degT1, 1.0)
                    srow = work.tile([1, N], fp32, tag="srow")
                    nc.scalar.add(srow[:, 0:128], degT0, 0.0)
                    nc.scalar.add(srow[:, 128:256], degT1, 0.0)
                    nc.scalar.dma_start(out=sums_p[g, 0:1], in_=srow)
                    rss[g] = rs
                # conv1 (rank-1 trick)
                hs = {}
                for g in gs:
                    p1 = ps.tile([128, N], fp32, tag="mm")
                    nc.tensor.matmul(p1, lhsT=s1row, rhs=rss[g].rearrange("a b c -> a (b c)"),
                                     start=True, stop=True)
                    hs[g] = p1
                for g in gs:
                    h = work.tile([128, N], fp32, tag="h")
                    nc.scalar.activation(h, hs[g], mybir.ActivationFunctionType.Relu, bias=Wb(0))
                    hs[g] = h
                for g in gs:
                    p2 = ps.tile([128, N], fp32, tag="mm")
                    nc.tensor.matmul(p2, lhsT=W(0), rhs=hs[g], start=True, stop=True)
                    hs[g] = p2
                for g in gs:
                    xt = work.tile([128, N], fp32, tag="xt")
                    nc.scalar.activation(xt, hs[g], mybir.ActivationFunctionType.Relu, bias=Wb(1))
                    xts[g] = xt

                # conv2, conv3
                for (wi1, wi2, bi1, bi2) in ((1, 2, 2, 3), (3, 4, 4, 5)):
                    xns, sps, aas = {}, {}, {}
                    for g in gs:
                        xnh = work.tile([128, 2, 128], bf16, tag="xnh")
                        xnl = work.tile([128, 2, 128], bf16, tag="xnl")
                        for c in range(2):
                            xtp = pst.tile([128, 128], fp32, tag="xtp")
                            nc.tensor.transpose(xtp, xts[g][:, 128 * c:128 * (c + 1)], identf)
                            nc.scalar.copy(xnh[:, c], xtp)
                            nc.vector.scalar_tensor_tensor(
                                xnl[:, c], in0=xtp, scalar=0.0, in1=xnh[:, c],
                                op0=mybir.AluOpType.add, op1=mybir.AluOpType.subtract)
                        xns[g] = (xnh, xnl)
                    for g in gs:
                        xnh, xnl = xns[g]
                        s_ps = ps.tile([128, N], fp32, tag="mm")
                        nc.tensor.matmul(s_ps, lhsT=xnh[:, 0], rhs=adjs[g][:, 0], start=True, stop=False)
                        nc.tensor.matmul(s_ps, lhsT=xnl[:, 0], rhs=adjs[g][:, 0], start=False, stop=False)
                        nc.tensor.matmul(s_ps, lhsT=xnh[:, 1], rhs=adjs[g][:, 1], start=False, stop=False)
                        nc.tensor.matmul(s_ps, lhsT=xnl[:, 1], rhs=adjs[g][:, 1], start=False, stop=True)
                        sps[g] = s_ps
                    for g in gs:
                        a = work.tile([128, N], fp32, tag="a")
                        nc.vector.scalar_tensor_tensor(
                            a, in0=sps[g], scalar=0.0, in1=xts[g],
                            op0=mybir.AluOpType.add, op1=mybir.AluOpType.add)
                        aas[g] = a
                    for g in gs:
                        pm1 = ps.tile([128, N], fp32, tag="mm")
                        nc.tensor.matmul(pm1, lhsT=W(wi1), rhs=aas[g], start=True, stop=True)
                        sps[g] = pm1
                    for g in gs:
                        h2 = work.tile([128, N], fp32, tag="h")
                        nc.scalar.activation(h2, sps[g], mybir.ActivationFunctionType.Relu, bias=Wb(bi1))
                        aas[g] = h2
                    for g in gs:
                        pm2 = ps.tile([128, N], fp32, tag="mm")
                        nc.tensor.matmul(pm2, lhsT=W(wi2), rhs=aas[g], start=True, stop=True)
                        sps[g] = pm2
                    for g in gs:
                        xt = work.tile([128, N], fp32, tag="xt")
                        nc.scalar.activation(xt, sps[g], mybir.ActivationFunctionType.Relu, bias=Wb(bi2))
                        xts[g] = xt
                for g in gs:
                    nc.scalar.dma_start(out=xt_p[g], in_=xts[g])
    nc.finalize()
    return nc


def _get_kernel():
    if "nc" not in _cache:
        _cache["nc"] = _build_bass()
    return _cache["nc"]


def _keys():
    if "k" not in _cache:
        import jax
        _cache["k"] = jax.random.split(jax.random.key(42), 3)
    return _cache["k"]


def _categorical(key, logits):
    """Sample exactly like the reference does (same env, same backend, same impl)."""
    import jax
    return np.asarray(jax.random.categorical(key, jax.numpy.asarray(logits), axis=-1))


def _np(t):
    return np.asarray(t, dtype=np.float32)


def _softmax(x):
    m = x.max(axis=-1, keepdims=True)
    e = np.exp(x - m)
    return e / e.sum(axis=-1, keepdims=True)


def kernel(states, params):
    from concourse.bass_utils import run_bass_kernel_spmd

    states = np.asarray(states, dtype=np.int32)
    p = {k: tuple(_np(t) for t in v) for k, v in params.items()}

    W1c1, b1c1, W2c1, b2c1 = p["conv1"]
    s1 = W1c1.sum(axis=0)  # ones(F_IN) @ W1c1
    wcols = [p["conv2"][0], p["conv2"][2], p["conv3"][0], p["conv3"][2]]
    wts = np.concatenate(
        [W2c1] + wcols
        + [b1c1[:, None], b2c1[:, None],
           p["conv2"][1][:, None], p["conv2"][3][:, None],
           p["conv3"][1][:, None], p["conv3"][3][:, None]], axis=1).astype(np.float32)
    wts = np.ascontiguousarray(wts)

    nc = _get_kernel()
    ones128 = np.ones((128, 128), np.float32)
    in_maps = [{"states": np.ascontiguousarray(states[i * GPC:(i + 1) * GPC]),
                "wts": wts, "s1row": np.ascontiguousarray(s1[None, :]),
                "ones": ones128}
               for i in range(NCORES)]
    _cache["in_maps"] = in_maps
    res = run_bass_kernel_spmd(nc, in_maps, core_ids=list(range(NCORES)))
    xt = np.concatenate([r["xt"] for r in res.results], axis=0)  # [G,128,N]
    sums = np.concatenate([r["sums"] for r in res.results], axis=0)  # [G,2,N]
    x = np.ascontiguousarray(xt.transpose(0, 2, 1))  # [G,N,E]
    colsum = sums[:, 1]

    # ---- host heads (fp32) ----
    valid = colsum > 0
    count = valid.sum(axis=1).astype(np.float32)
    rank = np.cumsum(valid, axis=1) - 1
    non_scaffold = valid & (rank < (count[:, None] - 1))

    def mlp2(h, pp):
        W1, b1, W2, b2 = pp
        return np.maximum(h @ W1 + b1, 0.0) @ W2 + b2

    kA, kB, kC = _keys()
    ninf = np.float32(-np.inf)

    logits_A = mlp2(x, p["mlp_A"])[..., 0]
    mlogits_A = np.where(non_scaffold, logits_A, ninf)
    probs_A = np.where(non_scaffold, _softmax(mlogits_A), 0.0).astype(np.float32)
    first = _categorical(kA, mlogits_A)

    first_emb = x[np.arange(G), first]  # [G,E]
    W1B, b1B, W2B, b2B = p["mlp_B"]
    hB = np.maximum(x @ W1B[:E] + first_emb[:, None, :] @ W1B[E:] + b1B, 0.0)
    logits_B = (hB @ W2B + b2B)[..., 0]
    mlogits_B = np.where(valid, logits_B, ninf)
    probs_B = np.where(valid, _softmax(mlogits_B), 0.0).astype(np.float32)
    second = _categorical(kB, mlogits_B)

    graph_emb = (x * valid[..., None]).sum(axis=1) / count[:, None]
    logits_C = mlp2(graph_emb, p["mlp_C"])
    probs_C = _softmax(logits_C).astype(np.float32)
    is_end = _categorical(kC, logits_C)

    action = np.stack([first, second, is_end], axis=1).astype(np.int32)
    return probs_A, probs_B, probs_C, action
